# revision 1
# baseline (speedup 1.0000x reference)
"""Trainium2 Bass kernel for nn_CrossAttention (16x512x64x64, 8 heads x 64).

Math notes (exact algebraic restructuring of the reference):
  The reference tiles ky=[b,1,1,c] to k=[b,c,1,c] before conv1x1(to_k_w), so
  every input channel of that conv carries the same value ky[b,j].  Hence
    conv1x1(k, to_k_w)[b,o,0,j] = rowsum(to_k_w)[o] * ky[b,j]     (rank-1)
  and likewise for v with rowsum(to_v_w) and vy.  Propagating this:
    ksm[b,hd,j] = softmax_j(rs_k[hd] * ky[b,j])
    w[b,hd]     = sum_j ksm[b,hd,j] * vy[b,j]
    context[b,h,d,e] = w[b,h,d] * rs_v[h,e]                        (rank-1)
    out[b,he,n] = rs_v[he] * s[b,h,n],  s = sum_d softmax_d(q)[d,n] * w[h,d]
    final[b,o,n] = sum_h W2[o,h] * s[b,h,n] + out_b[o],
      with W2[o,h] = scale * sum_e out_w[o, h*64+e] * rs_v[h*64+e]
  followed by GroupNorm(1) over (C,H,W) per sample.

  The only large compute left is q = to_q_w @ x (2.1 GFLOP/sample), computed
  transposed (qT[n,he] = x[c,n]^T @ to_q_wT[c,he]) so the d-softmax is a
  free-dim reduction over 64-wide head chunks.

Sharding: data-parallel over batch, 2 samples per core, 8 cores, no
collectives.  Each core gets the full weights.
"""

import numpy as np

import concourse.bass as bass
import concourse.mybir as mybir
import concourse.tile as tile
from concourse import bacc
from concourse.bass import ts
from concourse.bass_utils import run_bass_kernel_spmd

B, C, N = 16, 512, 4096
DIMY = 768
HEADS, DHEAD = 8, 64
NCORES = 8
BPC = B // NCORES  # samples per core
SCALE = DHEAD ** -0.5
EPS = 1e-5
F32 = mybir.dt.float32
F32R = mybir.dt.float32r
BF16 = mybir.dt.bfloat16
AX = mybir.AxisListType.X
AF = mybir.ActivationFunctionType
OP = mybir.AluOpType


def build_nc(use_f32r=True):
    MDT = F32R if use_f32r else F32

    nc = bacc.Bacc()
    xd = nc.dram_tensor("x", [BPC, C, N], F32, kind="ExternalInput")
    yd = nc.dram_tensor("y", [BPC, DIMY], F32, kind="ExternalInput")
    kwd = nc.dram_tensor("k_w", [C, DIMY], F32, kind="ExternalInput")
    vwd = nc.dram_tensor("v_w", [C, DIMY], F32, kind="ExternalInput")
    qwd = nc.dram_tensor("to_q_w", [C, C], F32, kind="ExternalInput")
    tkd = nc.dram_tensor("to_k_w", [C, C], F32, kind="ExternalInput")
    tvd = nc.dram_tensor("to_v_w", [C, C], F32, kind="ExternalInput")
    owd = nc.dram_tensor("out_w", [C, C], F32, kind="ExternalInput")
    obd = nc.dram_tensor("out_b", [C], F32, kind="ExternalInput")
    gngd = nc.dram_tensor("gn_g", [C], F32, kind="ExternalInput")
    gnbd = nc.dram_tensor("gn_b", [C], F32, kind="ExternalInput")
    outd = nc.dram_tensor("out", [BPC, C, N], F32, kind="ExternalOutput")

    from contextlib import ExitStack

    with tile.TileContext(nc) as tc, ExitStack() as ctx:
        persist = ctx.enter_context(tc.tile_pool(name="persist", bufs=1))
        prep = ctx.enter_context(tc.tile_pool(name="prep", bufs=1))
        bcastp = ctx.enter_context(tc.tile_pool(name="bcast", bufs=5))
        ezp = ctx.enter_context(tc.tile_pool(name="ezp", bufs=2))
        eqp = ctx.enter_context(tc.tile_pool(name="eqp", bufs=3))
        workp = ctx.enter_context(tc.tile_pool(name="workp", bufs=3))
        xp = ctx.enter_context(tc.tile_pool(name="xp", bufs=10))
        sttp = ctx.enter_context(tc.tile_pool(name="sttp", bufs=18))
        stgp = ctx.enter_context(tc.tile_pool(name="stgp", bufs=6))
        tep = ctx.enter_context(tc.tile_pool(name="tep", bufs=6))
        smallp = ctx.enter_context(tc.tile_pool(name="smallp", bufs=6))
        rowp = ctx.enter_context(tc.tile_pool(name="rowp", bufs=2))
        statsp = ctx.enter_context(tc.tile_pool(name="statsp", bufs=2))
        ybcp = ctx.enter_context(tc.tile_pool(name="ybcp", bufs=1))
        psqp = ctx.enter_context(tc.tile_pool(name="psqp", bufs=3, space="PSUM"))
        psfp = ctx.enter_context(tc.tile_pool(name="psfp", bufs=3, space="PSUM"))
        psf2p = psfp
        psmp = ctx.enter_context(tc.tile_pool(name="psmp", bufs=2, space="PSUM"))

        def bcast_row(src_row_ap, n, tag, dt=F32):
            """Broadcast a [1, n] SBUF row to [128, n] via a K=1 PE matmul
            against a ones row (internal-DRAM scratch fails to load here)."""
            ps_b = psmp.tile([128, n], F32, tag="pm")
            nc.tensor.matmul(ps_b, lhsT=ones_row, rhs=src_row_ap, start=True, stop=True)
            b = bcastp.tile([128, n], dt, tag="bc" if n == C else "bc_" + tag)
            nc.scalar.copy(out=b, in_=ps_b)
            return b

        # ---------------- prep (sample independent) ----------------
        ident = persist.tile([128, 128], F32, tag="ident")
        from concourse.masks import make_identity

        make_identity(nc, ident)
        ones_col = persist.tile([128, 1], F32, tag="ones")
        nc.vector.memset(ones_col, 1.0)
        ones_row = persist.tile([1, 128], F32, tag="onesr")
        nc.vector.memset(ones_row, 1.0)
        zero_col = persist.tile([128, 1], F32, tag="zero")
        nc.vector.memset(zero_col, 0.0)
        nc.const_aps.aps[(F32, 0.0)] = zero_col[:, :]
        eps_col = persist.tile([128, 1], F32, tag="eps")
        nc.vector.memset(eps_col, EPS)
        nc.const_aps.aps[(F32, EPS)] = eps_col[:, :]

        # per-o columns [128, 4]: col i holds values for o in [i*128,(i+1)*128)
        outb_col = persist.tile([128, 4], F32, tag="outb")
        nc.sync.dma_start(out=outb_col, in_=obd.rearrange("(i p) -> p i", p=128))
        gng_col = persist.tile([128, 4], F32, tag="gng")
        nc.sync.dma_start(out=gng_col, in_=gngd.rearrange("(i p) -> p i", p=128))
        gnb_col = persist.tile([128, 4], F32, tag="gnb")
        nc.sync.dma_start(out=gnb_col, in_=gnbd.rearrange("(i p) -> p i", p=128))

        # to_q_w transposed -> qwT[:, ct, :] = to_q_w.T[ct*128:(ct+1)*128, :]
        tq_nat = prep.tile([128, 4, DIMY], F32, tag="wnat")
        nc.sync.dma_start(
            out=tq_nat[:, :, :C], in_=qwd.rearrange("(i p) c -> p i c", p=128)
        )
        qwT = persist.tile([128, 4, C], BF16, tag="qwT")
        for ct in range(4):
            for ot in range(4):
                pst = psmp.tile([128, 128], F32, tag="pm")
                nc.tensor.transpose(pst, tq_nat[:, ot, ts(ct, 128)], ident)
                nc.scalar.copy(out=qwT[:, ct, ts(ot, 128)], in_=pst)

        # row sums of to_k_w / to_v_w  -> [128, 4] columns
        rsk_col = persist.tile([128, 4], F32, tag="rsk")
        rsv_col = persist.tile([128, 4], F32, tag="rsv")
        for dram, col in ((tkd, rsk_col), (tvd, rsv_col)):
            nat = prep.tile([128, 4, DIMY], F32, tag="wnat")
            nc.sync.dma_start(
                out=nat[:, :, :C], in_=dram.rearrange("(i p) c -> p i c", p=128)
            )
            for ot in range(4):
                nc.vector.reduce_sum(
                    out=col[:, ot : ot + 1], in_=nat[:, ot, :C], axis=AX
                )

        # rs_v as a broadcast row, scaled by softmax scale (folded into W2)
        ps_row = psmp.tile([1, C], F32, tag="pm")
        for ot in range(4):
            nc.tensor.transpose(
                ps_row[:, ts(ot, 128)], rsv_col[:, ot : ot + 1], ident
            )
        rsv_row = rowp.tile([1, C], F32, tag="rsvrow")
        nc.scalar.mul(out=rsv_row, in_=ps_row, mul=SCALE)
        rsv_b = bcast_row(rsv_row, C, "rsv")

        # W2T[h, ot, :]: W2[o,h] = sum_e out_w[o, h*64+e] * rs_v[h*64+e] * scale
        ow_nat = prep.tile([128, 4, DIMY], F32, tag="wnat")
        nc.sync.dma_start(
            out=ow_nat[:, :, :C], in_=owd.rearrange("(i p) c -> p i c", p=128)
        )
        w2T = persist.tile([HEADS, 4, 128], MDT, tag="w2T")
        for ot in range(4):
            t_ = workp.tile([128, C], F32, tag="tmp")
            nc.vector.tensor_mul(t_, ow_nat[:, ot, :C], rsv_b)
            w2c = smallp.tile([128, HEADS], F32, tag="w2c")
            nc.vector.reduce_sum(
                out=w2c, in_=t_.rearrange("p (h d) -> p h d", d=DHEAD), axis=AX
            )
            psw = psmp.tile([HEADS, 128], F32, tag="pm")
            nc.tensor.transpose(psw, w2c, ident)
            nc.scalar.copy(out=w2T[:, ot, :], in_=psw)

        # ky / vy rows per sample: ky[b,o] = sum_d y[b,d] * k_w[o,d]
        kyvy = persist.tile([1, 2 * BPC, C], F32, tag="kyvy")  # [kv*BPC+s]
        for kv, dram in ((0, kwd), (1, vwd)):
            nat = prep.tile([128, 4, DIMY], F32, tag="kvnat")
            nc.sync.dma_start(out=nat, in_=dram.rearrange("(i p) d -> p i d", p=128))
            for s in range(BPC):
                y_b = ybcp.tile([128, DIMY], F32, tag="yb")
                nc.gpsimd.dma_start(out=y_b, in_=yd[s].partition_broadcast(128))
                col = smallp.tile([128, 4], F32, tag="kycol")
                for ot in range(4):
                    scr = ybcp.tile([128, DIMY], F32, tag="yscr")
                    nc.vector.tensor_mul(scr, nat[:, ot, :], y_b)
                    nc.vector.reduce_sum(
                        out=col[:, ot : ot + 1], in_=scr, axis=AX
                    )
                psr = psmp.tile([1, C], F32, tag="pm")
                for ot in range(4):
                    nc.tensor.transpose(
                        psr[:, ts(ot, 128)], col[:, ot : ot + 1], ident
                    )
                nc.scalar.copy(out=kyvy[:, kv * BPC + s, :], in_=psr)

        # ---------------- per-sample main ----------------
        for s in range(BPC):
            ky_b = bcast_row(kyvy[:, s, :], C, "ky")
            vy_b = bcast_row(kyvy[:, BPC + s, :], C, "vy")

            # k-softmax + weighting: w[hd] = sum_j softmax_j(rs_k[hd]*ky[j]) vy[j]
            den_k = smallp.tile([128, 4], F32, tag="denk")
            num_k = smallp.tile([128, 4], F32, tag="numk")
            for t in range(4):
                ez = ezp.tile([128, C], F32, tag="ez")
                nc.scalar.activation(
                    out=ez,
                    in_=ky_b,
                    func=AF.Exp,
                    scale=rsk_col[:, t : t + 1],
                )
                nc.vector.reduce_sum(
                    out=den_k[:, t : t + 1], in_=ez, axis=AX
                )
                scr = workp.tile([128, C], F32, tag="tmp")
                nc.vector.tensor_mul(scr, ez, vy_b)
                nc.vector.reduce_sum(
                    out=num_k[:, t : t + 1], in_=scr, axis=AX
                )
            denr_k = smallp.tile([128, 4], F32, tag="denrk")
            nc.vector.reciprocal(denr_k, den_k)
            w_col = smallp.tile([128, 4], F32, tag="wcol")
            nc.vector.tensor_mul(w_col, num_k, denr_k)
            ps_w = psmp.tile([1, C], F32, tag="pm")
            for t in range(4):
                nc.tensor.transpose(ps_w[:, ts(t, 128)], w_col[:, t : t + 1], ident)
            w_row = rowp.tile([1, C], F32, tag="wrow")
            nc.scalar.copy(out=w_row, in_=ps_w)
            w_b = bcast_row(w_row, C, "w", dt=BF16)

            stats = statsp.tile([128, 4, 8, 6], F32, tag="stats")
            stt_tiles = []
            for g in range(8):  # n-groups of 512
                xcs = []
                for ct in range(4):
                    xc = xp.tile([128, 512], BF16, tag="xc")
                    nc.gpsimd.dma_start(
                        out=xc,
                        in_=xd[s, ts(ct, 128), ts(g, 512)],
                    )
                    xcs.append(xc)
                ps_stt = psmp.tile([HEADS, 512], F32, tag="pm")
                for j in range(4):  # n-tiles of 128 within the group
                    psq = psqp.tile([128, 512], F32, tag="psq")
                    for ct in range(4):
                        nc.tensor.matmul(
                            psq,
                            lhsT=xcs[ct][:, ts(j, 128)],
                            rhs=qwT[:, ct, :],
                            start=(ct == 0),
                            stop=(ct == 3),
                        )
                    te = tep.tile([128, 2, 512], BF16, tag="te")
                    nc.scalar.activation(out=te[:, 1, :], in_=psq, func=AF.Exp)
                    nc.gpsimd.tensor_mul(te[:, 0, :], te[:, 1, :], w_b)
                    sn2 = smallp.tile([128, 2, HEADS], F32, tag="sn2")
                    nc.vector.reduce_sum(
                        out=sn2,
                        in_=te.rearrange("p t (h d) -> p t h d", d=DHEAD),
                        axis=AX,
                    )
                    s_denr = smallp.tile([128, HEADS], F32, tag="sdenr")
                    nc.vector.reciprocal(s_denr, sn2[:, 1, :])
                    s_t = smallp.tile([128, HEADS], F32, tag="stile")
                    nc.vector.tensor_mul(s_t, sn2[:, 0, :], s_denr)
                    nc.tensor.transpose(ps_stt[:, ts(j, 128)], s_t, ident)
                stt = sttp.tile([HEADS, 512], MDT, tag="stt")
                nc.scalar.copy(out=stt, in_=ps_stt)
                stt_tiles.append(stt)
                for ot in range(4):
                    psf = psfp.tile([128, 512], F32, tag="psf")
                    nc.tensor.matmul(
                        psf,
                        lhsT=w2T[:, ot, :],
                        rhs=stt,
                        start=True,
                        stop=True,
                    )
                    nc.vector.bn_stats(out=stats[:, ot, g, :], in_=psf)

            # ---- GroupNorm(1) stats over the whole sample ----
            mvacc = smallp.tile([128, 2, 4], F32, tag="mvacc")
            for ot in range(4):
                mv = smallp.tile([128, 2], F32, tag="mv")
                nc.vector.bn_aggr(out=mv, in_=stats[:, ot, :, :])
                m_ = mvacc[:, 0, ot : ot + 1]
                nc.vector.tensor_add(m_, mv[:, 0:1], outb_col[:, ot : ot + 1])
                msq = smallp.tile([128, 1], F32, tag="msq")
                nc.vector.tensor_mul(msq, m_, m_)
                nc.vector.tensor_add(mvacc[:, 1, ot : ot + 1], mv[:, 1:2], msq)
            mv_tot = smallp.tile([128, 2], F32, tag="mvtot")
            nc.vector.reduce_sum(out=mv_tot, in_=mvacc, axis=AX)
            ps_tot = psmp.tile([1, 2], F32, tag="pm")
            nc.tensor.matmul(ps_tot, lhsT=ones_col, rhs=mv_tot, start=True, stop=True)
            tt = rowp.tile([1, 4], F32, tag="tt")
            nc.scalar.mul(out=tt[:, 0:2], in_=ps_tot, mul=1.0 / C)
            nc.vector.tensor_mul(tt[:, 2:3], tt[:, 0:1], tt[:, 0:1])  # mu^2
            nc.vector.tensor_sub(tt[:, 3:4], tt[:, 1:2], tt[:, 2:3])  # var
            sd = rowp.tile([1, 1], F32, tag="sd")
            nc.scalar.activation(out=sd, in_=tt[:, 3:4], func=AF.Sqrt, bias=EPS)
            rstd = rowp.tile([1, 1], F32, tag="rstd")
            nc.vector.reciprocal(rstd, sd)
            murow = rowp.tile([1, 2], F32, tag="mur")
            nc.vector.tensor_copy(murow[:, 0:1], tt[:, 0:1])
            nc.vector.tensor_copy(murow[:, 1:2], rstd)
            ms_b = bcast_row(murow, 2, "ms")

            # A = gn_g * rstd ; B = A*(out_b - mu) + gn_b ; out = A*mm + B
            a_col = smallp.tile([128, 4], F32, tag="acol")
            nc.vector.tensor_scalar_mul(a_col, gng_col, ms_b[:, 1:2])
            t1 = smallp.tile([128, 4], F32, tag="t1")
            nc.vector.tensor_scalar(
                out=t1, in0=outb_col, scalar1=ms_b[:, 0:1], scalar2=None,
                op0=OP.subtract,
            )
            t2 = smallp.tile([128, 4], F32, tag="t2")
            nc.vector.tensor_mul(t2, a_col, t1)
            b_col = smallp.tile([128, 4], F32, tag="bcol")
            nc.vector.tensor_add(b_col, t2, gnb_col)

            # rows: A and B2 as [1, 512] rows, A broadcast to 8 partitions
            ps_a = psmp.tile([1, C], F32, tag="pm")
            for ot in range(4):
                nc.tensor.transpose(
                    ps_a[:, ts(ot, 128)], a_col[:, ot : ot + 1], ident
                )
            a_row = rowp.tile([1, C], F32, tag="arow")
            nc.scalar.copy(out=a_row, in_=ps_a)
            ps_a8 = psmp.tile([HEADS, C], F32, tag="pm")
            nc.tensor.matmul(
                ps_a8,
                lhsT=ones_row[:, 0:HEADS],
                rhs=a_row,
                start=True,
                stop=True,
            )
            a8_sb = rowp.tile([HEADS, C], F32, tag="a8")
            nc.scalar.copy(out=a8_sb, in_=ps_a8)
            # w2s = W2T * A(o); B2(o) is added as bias in the staging copy
            w2s = rowp.tile([HEADS, 4, 128], MDT, tag="w2s")
            nc.vector.tensor_mul(
                w2s,
                w2T,
                a8_sb.rearrange("p (i f) -> p i f", i=4),
            )
            for g in range(8):
                for ot in range(4):
                    psf2 = psf2p.tile([128, 512], F32, tag="psf")
                    nc.tensor.matmul(
                        psf2,
                        lhsT=w2s[:, ot, :],
                        rhs=stt_tiles[g],
                        start=True,
                        stop=True,
                    )
                    stg = stgp.tile([128, 512], F32, tag="stg")
                    nc.scalar.activation(
                        out=stg,
                        in_=psf2,
                        func=AF.Identity,
                        bias=b_col[:, ot : ot + 1],
                    )
                    nc.sync.dma_start(
                        out=outd[s, ts(ot, 128), ts(g, 512)], in_=stg
                    )

    nc.finalize()
    return nc


_NC_CACHE = {}


def _get_nc(use_f32r=True):
    if use_f32r not in _NC_CACHE:
        _NC_CACHE[use_f32r] = build_nc(use_f32r)
    return _NC_CACHE[use_f32r]


def make_in_maps(inputs):
    x = np.ascontiguousarray(inputs["x"], dtype=np.float32).reshape(B, C, N)
    y = np.ascontiguousarray(inputs["y"], dtype=np.float32).reshape(B, DIMY)
    shared = {
        k: np.ascontiguousarray(inputs[k], dtype=np.float32)
        for k in (
            "k_w", "v_w", "to_q_w", "to_k_w", "to_v_w", "out_w",
            "out_b", "gn_g", "gn_b",
        )
    }
    in_maps = []
    for core in range(NCORES):
        s0 = core * BPC
        m = {"x": x[s0 : s0 + BPC], "y": y[s0 : s0 + BPC]}
        m.update(shared)
        in_maps.append(m)
    return in_maps


def kernel(**inputs):
    nc = _get_nc(use_f32r=True)
    res = run_bass_kernel_spmd(nc, make_in_maps(inputs), list(range(NCORES)))
    out = np.concatenate([r["out"] for r in res.results], axis=0)
    return out.reshape(B, C, 64, 64)


if __name__ == "__main__":
    rng = np.random.default_rng(0)
    inputs = {
        "x": rng.standard_normal((B, C, 64, 64), dtype=np.float32),
        "y": rng.standard_normal((B, 1, 1, DIMY), dtype=np.float32),
        "k_w": rng.standard_normal((C, DIMY), dtype=np.float32) * 0.02,
        "v_w": rng.standard_normal((C, DIMY), dtype=np.float32) * 0.02,
        "to_q_w": rng.standard_normal((C, C), dtype=np.float32) * 0.02,
        "to_k_w": rng.standard_normal((C, C), dtype=np.float32) * 0.02,
        "to_v_w": rng.standard_normal((C, C), dtype=np.float32) * 0.02,
        "out_w": rng.standard_normal((C, C), dtype=np.float32) * 0.02,
        "out_b": np.zeros(C, np.float32),
        "gn_g": np.ones(C, np.float32),
        "gn_b": np.zeros(C, np.float32),
    }
    out = kernel(**inputs)
    print("kernel ran, out shape", out.shape, "std", out.std())



# revision 4
# speedup vs baseline: 1.2159x; 1.2159x over previous
"""Trainium2 Bass kernel for nn_CrossAttention (16x512x64x64, 8 heads x 64).

Math notes (exact algebraic restructuring of the reference):
  The reference tiles ky=[b,1,1,c] to k=[b,c,1,c] before conv1x1(to_k_w), so
  every input channel of that conv carries the same value ky[b,j].  Hence
    conv1x1(k, to_k_w)[b,o,0,j] = rowsum(to_k_w)[o] * ky[b,j]     (rank-1)
  and likewise for v with rowsum(to_v_w) and vy.  Propagating this:
    ksm[b,hd,j] = softmax_j(rs_k[hd] * ky[b,j])
    w[b,hd]     = sum_j ksm[b,hd,j] * vy[b,j]
    s[b,h,n]    = sum_d softmax_d(q)[d,n] * w[h,d]
                = (sum_d w[hd] e^{q[hd,n]}) / (sum_d e^{q[hd,n]})
    final[b,o,n] = sum_h W2[o,h] * s[b,h,n] + out_b[o],
      with W2[o,h] = scale * sum_e out_w[o, h*64+e] * rs_v[h*64+e]
  followed by GroupNorm(1) over (C,H,W) per sample.

Kernel structure (per core = 2 samples, data-parallel over batch):
  - q computed in [he, n] orientation: psq[he_blk, n] = qwT_blk.T @ x_blk
    (host passes to_q_w pre-transposed, x pre-cast to bf16).
  - d-softmax numerator/denominator via mask MATMULs over the partition
    (he) dim: ndn = Mnum.T @ exp(psq), ndd = Hden.T @ exp(psq), where
    Hden is the static head-block 0/1 mask and Mnum = Hden * w[he].
  - s = ndn * reciprocal(ndd) on DVE (one fused op, accum_out gives the
    per-head row sums p1 for free).
  - GroupNorm stats WITHOUT a second big pass: mean from p1 (sum_n s),
    variance from the 8x8 Gram matrix S2 = s @ s.T (via PE transposes of
    s) and G = W2.T W2:  sum mm^2 = <G, S2>.
  - Final 512xN output = (A*W2).T @ s + B via small-K matmuls, with the
    GN affine folded into W2 (A) and the per-o bias (B) added during the
    PSUM->SBUF copy.  Output written as bf16, host upcasts.
"""

import numpy as np
import ml_dtypes

import concourse.bass as bass
import concourse.mybir as mybir
import concourse.tile as tile
from concourse import bacc
from concourse.bass import ts
from concourse.bass_utils import run_bass_kernel_spmd
from concourse.masks import make_identity

B, C, N = 16, 512, 4096
DIMY = 768
HEADS, DHEAD = 8, 64
NCORES = 8
BPC = B // NCORES  # samples per core
NG = 8             # n-groups per sample
GW = 512           # group width (pixels)
SCALE = DHEAD ** -0.5
EPS = 1e-5
M_TOT = float(C * N)
F32 = mybir.dt.float32
BF16 = mybir.dt.bfloat16
AX = mybir.AxisListType.X
AF = mybir.ActivationFunctionType
OP = mybir.AluOpType
NPBF = ml_dtypes.bfloat16


def build_nc(use_f32r=True):
    nc = bacc.Bacc()
    xd = nc.dram_tensor("x", [BPC, C, N], BF16, kind="ExternalInput")
    yd = nc.dram_tensor("y", [BPC, DIMY], BF16, kind="ExternalInput")
    kwTd = nc.dram_tensor("k_wT", [DIMY, C], BF16, kind="ExternalInput")
    vwTd = nc.dram_tensor("v_wT", [DIMY, C], BF16, kind="ExternalInput")
    qwTd = nc.dram_tensor("to_q_wT", [C, C], BF16, kind="ExternalInput")
    tkd = nc.dram_tensor("to_k_w", [C, C], BF16, kind="ExternalInput")
    tvd = nc.dram_tensor("to_v_w", [C, C], BF16, kind="ExternalInput")
    owd = nc.dram_tensor("out_w", [C, C], BF16, kind="ExternalInput")
    obd = nc.dram_tensor("out_b", [C], F32, kind="ExternalInput")
    gngd = nc.dram_tensor("gn_g", [C], F32, kind="ExternalInput")
    gnbd = nc.dram_tensor("gn_b", [C], F32, kind="ExternalInput")
    outd = nc.dram_tensor("out", [BPC, C, N], BF16, kind="ExternalOutput")

    from contextlib import ExitStack

    with tile.TileContext(nc) as tc, ExitStack() as ctx:
        persist = ctx.enter_context(tc.tile_pool(name="persist", bufs=1))
        prep = ctx.enter_context(tc.tile_pool(name="prep", bufs=2))
        workp = ctx.enter_context(tc.tile_pool(name="workp", bufs=2))
        smallp = ctx.enter_context(tc.tile_pool(name="smallp", bufs=2))
        samp = ctx.enter_context(tc.tile_pool(name="samp", bufs=2))
        rowp = ctx.enter_context(tc.tile_pool(name="rowp", bufs=2))
        ezp = ctx.enter_context(tc.tile_pool(name="ezp", bufs=2))
        xp = ctx.enter_context(tc.tile_pool(name="xp", bufs=3))
        ep = ctx.enter_context(tc.tile_pool(name="ep", bufs=5))
        sttp = ctx.enter_context(tc.tile_pool(name="sttp", bufs=17))
        stap = ctx.enter_context(tc.tile_pool(name="stap", bufs=2))
        rcpp = ctx.enter_context(tc.tile_pool(name="rcpp", bufs=2))
        stgp = ctx.enter_context(tc.tile_pool(name="stgp", bufs=3))
        # PSUM: 8 banks total
        psqp = ctx.enter_context(tc.tile_pool(name="psqp", bufs=2, space="PSUM"))
        ndnp = ctx.enter_context(tc.tile_pool(name="ndnp", bufs=1, space="PSUM"))
        nddp = ctx.enter_context(tc.tile_pool(name="nddp", bufs=1, space="PSUM"))
        pstp = ctx.enter_context(tc.tile_pool(name="pstp", bufs=1, space="PSUM"))
        psfp = ctx.enter_context(tc.tile_pool(name="psfp", bufs=2, space="PSUM"))
        psmp = ctx.enter_context(tc.tile_pool(name="psmp", bufs=1, space="PSUM"))

        # ---------------- constants ----------------
        ident = persist.tile([128, 128], F32, tag="ident")
        make_identity(nc, ident)
        identB = persist.tile([128, 128], BF16, tag="identB")
        make_identity(nc, identB)
        ones_row = persist.tile([1, 128], F32, tag="onesr")
        nc.vector.memset(ones_row, 1.0)
        ones_col = persist.tile([128, 1], F32, tag="onesc")
        nc.vector.memset(ones_col, 1.0)
        ones8 = persist.tile([8, 1], F32, tag="ones8")
        nc.vector.memset(ones8, 1.0)
        zero_col = persist.tile([128, 1], F32, tag="zero")
        nc.vector.memset(zero_col, 0.0)
        nc.const_aps.aps[(F32, 0.0)] = zero_col[:, :]
        eps_col = persist.tile([128, 1], F32, tag="eps")
        nc.vector.memset(eps_col, EPS)
        nc.const_aps.aps[(F32, EPS)] = eps_col[:, :]

        outb_col = persist.tile([128, 4], F32, tag="outb")
        nc.sync.dma_start(out=outb_col, in_=obd.rearrange("(i p) -> p i", p=128))
        gng_col = persist.tile([128, 4], F32, tag="gng")
        nc.sync.dma_start(out=gng_col, in_=gngd.rearrange("(i p) -> p i", p=128))
        gnb_col = persist.tile([128, 4], F32, tag="gnb")
        nc.sync.dma_start(out=gnb_col, in_=gnbd.rearrange("(i p) -> p i", p=128))

        # weights (host pre-transposed where needed)
        qwT_sb = persist.tile([128, 4, C], BF16, tag="qwT")
        nc.sync.dma_start(out=qwT_sb, in_=qwTd.rearrange("(i p) o -> p i o", p=128))
        kwT_sb = persist.tile([128, 6, C], BF16, tag="kwT")
        nc.sync.dma_start(out=kwT_sb, in_=kwTd.rearrange("(c p) o -> p c o", p=128))
        vwT_sb = persist.tile([128, 6, C], BF16, tag="vwT")
        nc.sync.dma_start(out=vwT_sb, in_=vwTd.rearrange("(c p) o -> p c o", p=128))

        # row sums of to_k_w / to_v_w (he-layout columns)
        rsk_col = persist.tile([128, 4], F32, tag="rsk")
        rsv_col = persist.tile([128, 4], F32, tag="rsv")
        for dram, col in ((tkd, rsk_col), (tvd, rsv_col)):
            nat = prep.tile([128, 4, C], BF16, tag="wnat")
            nc.sync.dma_start(out=nat, in_=dram.rearrange("(i p) c -> p i c", p=128))
            nc.vector.reduce_sum(out=col, in_=nat, axis=AX)

        # rs_v as a broadcast row scaled by softmax scale
        ps_r = psmp.tile([1, C], F32, tag="pm")
        for ot in range(4):
            nc.tensor.transpose(ps_r[:, ts(ot, 128)], rsv_col[:, ot : ot + 1], ident)
        rsv_row = rowp.tile([1, C], F32, tag="rsvrow")
        nc.vector.tensor_scalar_mul(rsv_row, ps_r, SCALE)
        ps_rb = psmp.tile([128, C], F32, tag="pm")
        nc.tensor.matmul(ps_rb, lhsT=ones_row, rhs=rsv_row, start=True, stop=True)

        # W2 (o-major cols) and its transpose blocks
        ow_nat = prep.tile([128, 4, C], BF16, tag="wnat")
        nc.sync.dma_start(out=ow_nat, in_=owd.rearrange("(i p) c -> p i c", p=128))
        w2c = persist.tile([128, 4, HEADS], F32, tag="w2c")
        for ot in range(4):
            t_ = workp.tile([128, C], F32, tag="tmp")
            nc.vector.tensor_mul(t_, ow_nat[:, ot, :], ps_rb)
            nc.vector.reduce_sum(
                out=w2c[:, ot, :],
                in_=t_.rearrange("p (h d) -> p h d", d=DHEAD),
                axis=AX,
            )
        w2T = persist.tile([HEADS, 4, 128], BF16, tag="w2T")
        for ot in range(4):
            psw = psmp.tile([HEADS, 128], F32, tag="pm")
            nc.tensor.transpose(psw, w2c[:, ot, :], ident)
            nc.vector.tensor_copy(w2T[:, ot, :], psw)
        w2cb = persist.tile([128, 4, HEADS], BF16, tag="w2cb")
        nc.vector.tensor_copy(w2cb, w2c)

        # G = W2^T W2 ; rsW2 = W2^T 1 ; wb = W2^T out_b
        obo4 = persist.tile([128, 4, 2], BF16, tag="obo4")
        nc.vector.memset(obo4, 1.0)
        for ot in range(4):
            nc.vector.tensor_copy(obo4[:, ot, 1:2], outb_col[:, ot : ot + 1])
        ps_rw = psmp.tile([HEADS, 2], F32, tag="pm")
        for ot in range(4):
            nc.tensor.matmul(
                ps_rw, lhsT=w2cb[:, ot, :], rhs=obo4[:, ot, :],
                start=(ot == 0), stop=(ot == 3),
            )
        rwb = persist.tile([HEADS, 2], F32, tag="rwb")
        nc.vector.tensor_copy(rwb, ps_rw)
        ps_g = psmp.tile([HEADS, HEADS], F32, tag="pm")
        for ot in range(4):
            nc.tensor.matmul(
                ps_g, lhsT=w2cb[:, ot, :], rhs=w2cb[:, ot, :],
                start=(ot == 0), stop=(ot == 3),
            )
        Gt = persist.tile([HEADS, HEADS], F32, tag="Gt")
        nc.vector.tensor_copy(Gt, ps_g)

        # sum(out_b), sum(out_b^2) scalars
        ob2 = smallp.tile([128, 4], F32, tag="ob2")
        nc.vector.tensor_mul(ob2, outb_col, outb_col)
        ps_o = psmp.tile([1, 8], F32, tag="pm")
        nc.tensor.matmul(ps_o[:, 0:4], lhsT=ones_col, rhs=outb_col, start=True, stop=True)
        nc.tensor.matmul(ps_o[:, 4:8], lhsT=ones_col, rhs=ob2, start=True, stop=True)
        obsums = rowp.tile([1, 8], F32, tag="obsums")
        nc.vector.tensor_copy(obsums, ps_o)
        obs = persist.tile([1, 2], F32, tag="obs")
        nc.vector.reduce_sum(obs[:, 0:1], obsums[:, 0:4], axis=AX)
        nc.vector.reduce_sum(obs[:, 1:2], obsums[:, 4:8], axis=AX)

        # static head-block mask: Hden[p, ot, h] = 1 iff h == 2*ot + p//64
        Hden = persist.tile([128, 4, HEADS], BF16, tag="Hden")
        nc.vector.memset(Hden, 0.0)
        for ot in range(4):
            nc.vector.memset(Hden[0:64, ot, 2 * ot : 2 * ot + 1], 1.0)
            nc.vector.memset(Hden[64:128, ot, 2 * ot + 1 : 2 * ot + 2], 1.0)

        # ---------------- per-sample pieces ----------------
        def sample_head(s):
            """ky/vy, k-softmax -> w, Mnum mask; returns per-sample tiles."""
            y_col = smallp.tile([128, 6], BF16, tag="ycol")
            nc.sync.dma_start(out=y_col, in_=yd[s].rearrange("(c p) -> p c", p=128))
            rows = {}
            for tag, wsb in (("ky", kwT_sb), ("vy", vwT_sb)):
                ps_k = psmp.tile([1, C], F32, tag="pm")
                for c in range(6):
                    nc.tensor.matmul(
                        ps_k, lhsT=y_col[:, c : c + 1], rhs=wsb[:, c, :],
                        start=(c == 0), stop=(c == 5),
                    )
                r = rowp.tile([1, C], F32, tag=tag + "row")
                nc.vector.tensor_copy(r, ps_k)
                rows[tag] = r
            ps_vb = psmp.tile([128, C], F32, tag="pm")
            nc.tensor.matmul(ps_vb, lhsT=ones_row, rhs=rows["vy"], start=True, stop=True)
            vyb = samp.tile([128, C], F32, tag="vyb")
            nc.vector.tensor_copy(vyb, ps_vb)
            ps_kb = psmp.tile([128, C], F32, tag="pm")
            nc.tensor.matmul(ps_kb, lhsT=ones_row, rhs=rows["ky"], start=True, stop=True)
            den4 = samp.tile([128, 4], F32, tag="den4")
            num4 = samp.tile([128, 4], F32, tag="num4")
            for t in range(4):
                ez = ezp.tile([128, C], BF16, tag="ez")
                nc.scalar.activation(
                    out=ez, in_=ps_kb, func=AF.Exp,
                    scale=rsk_col[:, t : t + 1],
                    accum_out=den4[:, t : t + 1],
                )
                scr = ezp.tile([128, C], BF16, tag="scr")
                nc.vector.scalar_tensor_tensor(
                    out=scr, in0=ez, scalar=1.0, in1=vyb,
                    op0=OP.mult, op1=OP.mult,
                    accum_out=num4[:, t : t + 1],
                )
            rcp4 = samp.tile([128, 4], F32, tag="rcp4")
            nc.vector.reciprocal(rcp4, den4)
            wcol = samp.tile([128, 4], F32, tag="wcol")
            nc.vector.tensor_mul(wcol, num4, rcp4)
            Mnum = samp.tile([128, 4, HEADS], BF16, tag="mnum")
            for ot in range(4):
                nc.vector.tensor_scalar_mul(
                    Mnum[:, ot, :], Hden[:, ot, :], wcol[:, ot : ot + 1]
                )
            S2all = samp.tile([HEADS, NG, HEADS], F32, tag="s2all")
            p1a = samp.tile([HEADS, NG], F32, tag="p1a")
            return {"Mnum": Mnum, "S2all": S2all, "p1a": p1a}

        def phase1_g(s, st, g):
            xg = xp.tile([128, 4, GW], BF16, tag="xg")
            nc.sync.dma_start(
                out=xg,
                in_=xd[s].rearrange("(i p) n -> p i n", p=128)[:, :, ts(g, GW)],
            )
            Es = []
            for ot in range(4):
                psq = psqp.tile([128, GW], F32, tag="psq")
                for ct in range(4):
                    nc.tensor.matmul(
                        psq, lhsT=qwT_sb[:, ct, ts(ot, 128)], rhs=xg[:, ct, :],
                        start=(ct == 0), stop=(ct == 3),
                    )
                E = ep.tile([128, GW], BF16, tag="E")
                nc.scalar.activation(out=E, in_=psq, func=AF.Exp)
                Es.append(E)
            ndn = ndnp.tile([HEADS, GW], F32, tag="ndn")
            ndd = nddp.tile([HEADS, GW], F32, tag="ndd")
            for ot in range(4):
                nc.tensor.matmul(
                    ndn, lhsT=st["Mnum"][:, ot, :], rhs=Es[ot],
                    start=(ot == 0), stop=(ot == 3),
                )
                nc.tensor.matmul(
                    ndd, lhsT=Hden[:, ot, :], rhs=Es[ot],
                    start=(ot == 0), stop=(ot == 3),
                )
            rcp8 = rcpp.tile([HEADS, GW], F32, tag="rcp")
            nc.vector.reciprocal(rcp8, ndd)
            stt = sttp.tile([HEADS, GW], BF16, tag="stt")
            nc.vector.scalar_tensor_tensor(
                out=stt, in0=ndn, scalar=1.0, in1=rcp8,
                op0=OP.mult, op1=OP.mult,
                accum_out=st["p1a"][:, g : g + 1],
            )
            # Gram contribution: transpose s -> [n, h] chunks, S2g = sT.T @ sT
            pst = pstp.tile([128, 32], BF16, tag="pst")
            for j in range(4):
                nc.tensor.transpose(
                    pst[:, ts(j, 8)], stt[:, ts(j, 128)], identB[0:8, 0:8]
                )
            sta = stap.tile([128, 32], BF16, tag="sta")
            nc.vector.tensor_copy(sta, pst)
            psg = psmp.tile([HEADS, HEADS], F32, tag="pm")
            for j in range(4):
                nc.tensor.matmul(
                    psg, lhsT=sta[:, ts(j, 8)], rhs=sta[:, ts(j, 8)],
                    start=(j == 0), stop=(j == 3),
                )
            nc.vector.tensor_copy(st["S2all"][:, g, :], psg)
            return stt

        def sample_stats(s, st):
            p1 = samp.tile([HEADS, 1], F32, tag="p1")
            nc.vector.reduce_sum(p1, st["p1a"], axis=AX)
            S2s = samp.tile([HEADS, HEADS], F32, tag="s2s")
            nc.vector.reduce_sum(
                S2s, st["S2all"].rearrange("p g h -> p h g"), axis=AX
            )
            tmp3 = samp.tile([HEADS, 3], F32, tag="t3")
            nc.vector.tensor_mul(tmp3[:, 0:1], rwb[:, 0:1], p1)
            nc.vector.tensor_mul(tmp3[:, 2:3], rwb[:, 1:2], p1)
            gs = samp.tile([HEADS, HEADS], F32, tag="gs")
            nc.vector.tensor_mul(gs, Gt, S2s)
            nc.vector.reduce_sum(tmp3[:, 1:2], gs, axis=AX)
            ps_t = psmp.tile([1, 3], F32, tag="pm")
            nc.tensor.matmul(ps_t, lhsT=ones8, rhs=tmp3, start=True, stop=True)
            tt = rowp.tile([1, 12], F32, tag="tt")
            nc.vector.tensor_copy(tt[:, 0:3], ps_t)
            # mu = (sum_mm + N*sum_ob) / M
            nc.vector.scalar_tensor_tensor(
                out=tt[:, 3:4], in0=obs[:, 0:1], scalar=float(N), in1=tt[:, 0:1],
                op0=OP.mult, op1=OP.add,
            )
            nc.vector.tensor_scalar_mul(tt[:, 4:5], tt[:, 3:4], 1.0 / M_TOT)
            # e2 = (sumsq_mm + 2*wb.p1 + N*ssq_ob) / M
            nc.vector.scalar_tensor_tensor(
                out=tt[:, 5:6], in0=tt[:, 2:3], scalar=2.0, in1=tt[:, 1:2],
                op0=OP.mult, op1=OP.add,
            )
            nc.vector.scalar_tensor_tensor(
                out=tt[:, 6:7], in0=obs[:, 1:2], scalar=float(N), in1=tt[:, 5:6],
                op0=OP.mult, op1=OP.add,
            )
            nc.vector.tensor_scalar_mul(tt[:, 7:8], tt[:, 6:7], 1.0 / M_TOT)
            nc.vector.tensor_mul(tt[:, 8:9], tt[:, 4:5], tt[:, 4:5])   # mu^2
            nc.vector.tensor_sub(tt[:, 9:10], tt[:, 7:8], tt[:, 8:9])  # var
            nc.scalar.activation(out=tt[:, 10:11], in_=tt[:, 9:10], func=AF.Sqrt, bias=EPS)
            nc.vector.reciprocal(tt[:, 11:12], tt[:, 10:11])           # rstd
            murow = rowp.tile([1, 2], F32, tag="mur")
            nc.vector.tensor_copy(murow[:, 0:1], tt[:, 4:5])
            nc.vector.tensor_copy(murow[:, 1:2], tt[:, 11:12])
            ps_ms = psmp.tile([128, 2], F32, tag="pm")
            nc.tensor.matmul(ps_ms, lhsT=ones_row, rhs=murow, start=True, stop=True)
            msb = samp.tile([128, 2], F32, tag="msb")
            nc.vector.tensor_copy(msb, ps_ms)
            Acol = samp.tile([128, 4], F32, tag="acol")
            nc.vector.tensor_scalar_mul(Acol, gng_col, msb[:, 1:2])
            tb1 = samp.tile([128, 4], F32, tag="tb1")
            nc.vector.tensor_scalar(
                out=tb1, in0=outb_col, scalar1=msb[:, 0:1], scalar2=None,
                op0=OP.subtract,
            )
            tb2 = samp.tile([128, 4], F32, tag="tb2")
            nc.vector.tensor_mul(tb2, Acol, tb1)
            Bcol = samp.tile([128, 4], F32, tag="bcol")
            nc.vector.tensor_add(Bcol, tb2, gnb_col)
            ps_a = psmp.tile([1, C], F32, tag="pm")
            for ot in range(4):
                nc.tensor.transpose(ps_a[:, ts(ot, 128)], Acol[:, ot : ot + 1], ident)
            a_row = rowp.tile([1, C], F32, tag="arow")
            nc.vector.tensor_copy(a_row, ps_a)
            ps_a8 = psmp.tile([HEADS, C], F32, tag="pm")
            nc.tensor.matmul(
                ps_a8, lhsT=ones_row[:, 0:HEADS], rhs=a_row, start=True, stop=True
            )
            a8 = samp.tile([HEADS, C], F32, tag="a8")
            nc.vector.tensor_copy(a8, ps_a8)
            w2sT = samp.tile([HEADS, 4, 128], BF16, tag="w2s")
            nc.vector.tensor_mul(w2sT, w2T, a8.rearrange("p (i f) -> p i f", i=4))
            return {"w2sT": w2sT, "Bcol": Bcol}

        def phase2_g(s, fin, stt, g):
            stg = stgp.tile([128, 4, GW], BF16, tag="stg")
            for ot in range(4):
                psf = psfp.tile([128, GW], F32, tag="psf")
                nc.tensor.matmul(
                    psf, lhsT=fin["w2sT"][:, ot, :], rhs=stt, start=True, stop=True
                )
                if ot % 2 == 0:
                    nc.vector.tensor_scalar_add(
                        stg[:, ot, :], psf, fin["Bcol"][:, ot : ot + 1]
                    )
                else:
                    nc.scalar.activation(
                        out=stg[:, ot, :], in_=psf, func=AF.Identity,
                        bias=fin["Bcol"][:, ot : ot + 1],
                    )
            nc.gpsimd.dma_start(
                out=outd[s].rearrange("(i p) n -> p i n", p=128)[:, :, ts(g, GW)],
                in_=stg,
            )

        # ---------------- schedule ----------------
        st0 = sample_head(0)
        stt0 = [phase1_g(0, st0, g) for g in range(NG)]
        fin0 = sample_stats(0, st0)
        st1 = sample_head(1)
        stt1 = []
        for g in range(NG):
            stt1.append(phase1_g(1, st1, g))
            phase2_g(0, fin0, stt0[g], g)
        fin1 = sample_stats(1, st1)
        for g in range(NG):
            phase2_g(1, fin1, stt1[g], g)

    nc.finalize()
    return nc


_NC_CACHE = {}


def _get_nc(use_f32r=True):
    if use_f32r not in _NC_CACHE:
        _NC_CACHE[use_f32r] = build_nc(use_f32r)
    return _NC_CACHE[use_f32r]


def make_in_maps(inputs):
    x = np.ascontiguousarray(inputs["x"], dtype=np.float32).reshape(B, C, N)
    x = x.astype(NPBF)
    y = np.asarray(inputs["y"], dtype=np.float32).reshape(B, DIMY).astype(NPBF)
    f32 = lambda k: np.asarray(inputs[k], dtype=np.float32)
    shared = {
        "k_wT": f32("k_w").T.astype(NPBF),
        "v_wT": f32("v_w").T.astype(NPBF),
        "to_q_wT": f32("to_q_w").T.astype(NPBF),
        "to_k_w": f32("to_k_w").astype(NPBF),
        "to_v_w": f32("to_v_w").astype(NPBF),
        "out_w": f32("out_w").astype(NPBF),
        "out_b": f32("out_b"),
        "gn_g": f32("gn_g"),
        "gn_b": f32("gn_b"),
    }
    in_maps = []
    for core in range(NCORES):
        s0 = core * BPC
        m = {"x": x[s0 : s0 + BPC], "y": y[s0 : s0 + BPC]}
        m.update(shared)
        in_maps.append(m)
    return in_maps


def kernel(**inputs):
    nc = _get_nc(use_f32r=True)
    res = run_bass_kernel_spmd(nc, make_in_maps(inputs), list(range(NCORES)))
    out = np.concatenate([r["out"] for r in res.results], axis=0)
    return out.astype(np.float32).reshape(B, C, 64, 64)


if __name__ == "__main__":
    rng = np.random.default_rng(0)
    inputs = {
        "x": rng.standard_normal((B, C, 64, 64), dtype=np.float32),
        "y": rng.standard_normal((B, 1, 1, DIMY), dtype=np.float32),
        "k_w": rng.standard_normal((C, DIMY), dtype=np.float32) * 0.02,
        "v_w": rng.standard_normal((C, DIMY), dtype=np.float32) * 0.02,
        "to_q_w": rng.standard_normal((C, C), dtype=np.float32) * 0.02,
        "to_k_w": rng.standard_normal((C, C), dtype=np.float32) * 0.02,
        "to_v_w": rng.standard_normal((C, C), dtype=np.float32) * 0.02,
        "out_w": rng.standard_normal((C, C), dtype=np.float32) * 0.02,
        "out_b": np.zeros(C, np.float32),
        "gn_g": np.ones(C, np.float32),
        "gn_b": np.zeros(C, np.float32),
    }
    out = kernel(**inputs)
    print("kernel ran, out shape", out.shape, "std", out.std())


# revision 12
# speedup vs baseline: 1.5819x; 1.3010x over previous
"""Trainium2 Bass kernel for nn_CrossAttention (16x512x64x64, 8 heads x 64).

Math notes (exact algebraic restructuring of the reference):
  The reference tiles ky=[b,1,1,c] to k=[b,c,1,c] before conv1x1(to_k_w), so
  every input channel of that conv carries the same value ky[b,j].  Hence
    conv1x1(k, to_k_w)[b,o,0,j] = rowsum(to_k_w)[o] * ky[b,j]     (rank-1)
  and likewise for v with rowsum(to_v_w) and vy.  Propagating this:
    ksm[b,hd,j] = softmax_j(rs_k[hd] * ky[b,j])
    w[b,hd]     = sum_j ksm[b,hd,j] * vy[b,j]
    s[b,h,n]    = sum_d softmax_d(q)[d,n] * w[h,d]
                = (sum_d w[hd] e^{q[hd,n]}) / (sum_d e^{q[hd,n]})
    final[b,o,n] = sum_h W2[o,h] * s[b,h,n] + out_b[o],
      with W2[o,h] = scale * sum_e out_w[o, h*64+e] * rs_v[h*64+e]
  followed by GroupNorm(1) over (C,H,W) per sample.

Kernel structure (per core = 2 samples, data-parallel over batch):
  - q computed in [he, n] orientation: psq[he_blk, n] = qwT_blk.T @ x_blk
    (host passes to_q_w pre-transposed, x pre-cast to bf16).
  - d-softmax numerator/denominator via mask MATMULs over the partition
    (he) dim: ndn = Mnum.T @ exp(psq), ndd = Hden.T @ exp(psq), where
    Hden is the static head-block 0/1 mask and Mnum = Hden * w[he].
  - s = ndn * reciprocal(ndd) on DVE (one fused op, accum_out gives the
    per-head row sums p1 for free).
  - GroupNorm stats WITHOUT a second big pass: mean from p1 (sum_n s),
    variance from the 8x8 Gram matrix S2 = s @ s.T (via PE transposes of
    s) and G = W2.T W2:  sum mm^2 = <G, S2>.
  - Final 512xN output = (A*W2).T @ s + B via small-K matmuls, with the
    GN affine folded into W2 (A) and the per-o bias (B) added during the
    PSUM->SBUF copy.  Output written as bf16, host upcasts.
"""

import numpy as np
import ml_dtypes

import concourse.bass as bass
import concourse.mybir as mybir
import concourse.tile as tile
from concourse import bacc
from concourse.bass import ts
from concourse.bass_utils import run_bass_kernel_spmd
from concourse.masks import make_identity

B, C, N = 16, 512, 4096
DIMY = 768
HEADS, DHEAD = 8, 64
NCORES = 8
BPC = B // NCORES  # samples per core
NG = 8             # n-groups per sample
GW = 512           # group width (pixels)
SCALE = DHEAD ** -0.5
EPS = 1e-5
M_TOT = float(C * N)
F32 = mybir.dt.float32
BF16 = mybir.dt.bfloat16
AX = mybir.AxisListType.X
AF = mybir.ActivationFunctionType
OP = mybir.AluOpType
NPBF = ml_dtypes.bfloat16


def build_nc(use_f32r=True):
    nc = bacc.Bacc()
    xd = nc.dram_tensor("x", [BPC, C, N], BF16, kind="ExternalInput")
    yd = nc.dram_tensor("y", [BPC, DIMY], BF16, kind="ExternalInput")
    kwTd = nc.dram_tensor("k_wT", [DIMY, C], BF16, kind="ExternalInput")
    vwTd = nc.dram_tensor("v_wT", [DIMY, C], BF16, kind="ExternalInput")
    qwTd = nc.dram_tensor("to_q_wT", [C, C], BF16, kind="ExternalInput")
    tkd = nc.dram_tensor("to_k_w", [C, C], BF16, kind="ExternalInput")
    tvd = nc.dram_tensor("to_v_w", [C, C], BF16, kind="ExternalInput")
    owd = nc.dram_tensor("out_w", [C, C], BF16, kind="ExternalInput")
    obd = nc.dram_tensor("out_b", [C], F32, kind="ExternalInput")
    gngd = nc.dram_tensor("gn_g", [C], F32, kind="ExternalInput")
    gnbd = nc.dram_tensor("gn_b", [C], F32, kind="ExternalInput")
    outd = nc.dram_tensor("out", [BPC, C, N], BF16, kind="ExternalOutput")

    from contextlib import ExitStack

    with tile.TileContext(nc) as tc, ExitStack() as ctx:
        persist = ctx.enter_context(tc.tile_pool(name="persist", bufs=1))
        prep = ctx.enter_context(tc.tile_pool(name="prep", bufs=2))
        workp = ctx.enter_context(tc.tile_pool(name="workp", bufs=2))
        smallp = ctx.enter_context(tc.tile_pool(name="smallp", bufs=2))
        samp = ctx.enter_context(tc.tile_pool(name="samp", bufs=2))
        rowp = ctx.enter_context(tc.tile_pool(name="rowp", bufs=2))
        ezp = ctx.enter_context(tc.tile_pool(name="ezp", bufs=2))
        xp = ctx.enter_context(tc.tile_pool(name="xp", bufs=3))
        ep = ctx.enter_context(tc.tile_pool(name="ep", bufs=5))
        sttp = ctx.enter_context(tc.tile_pool(name="sttp", bufs=17))
        stap = ctx.enter_context(tc.tile_pool(name="stap", bufs=2))
        rcpp = ctx.enter_context(tc.tile_pool(name="rcpp", bufs=2))
        stgp = ctx.enter_context(tc.tile_pool(name="stgp", bufs=3))
        # PSUM: 8 banks total
        psqp = ctx.enter_context(tc.tile_pool(name="psqp", bufs=2, space="PSUM"))
        ndnp = ctx.enter_context(tc.tile_pool(name="ndnp", bufs=1, space="PSUM"))
        nddp = ctx.enter_context(tc.tile_pool(name="nddp", bufs=1, space="PSUM"))
        pstp = ctx.enter_context(tc.tile_pool(name="pstp", bufs=1, space="PSUM"))
        psfp = ctx.enter_context(tc.tile_pool(name="psfp", bufs=2, space="PSUM"))
        psmp = ctx.enter_context(tc.tile_pool(name="psmp", bufs=1, space="PSUM"))

        # ---------------- constants ----------------
        ident = persist.tile([128, 128], F32, tag="ident")
        make_identity(nc, ident)
        identB = persist.tile([128, 128], BF16, tag="identB")
        make_identity(nc, identB)
        ones_row = persist.tile([1, 128], F32, tag="onesr")
        nc.vector.memset(ones_row, 1.0)
        ones_rowB = persist.tile([1, 128], BF16, tag="onesrB")
        nc.vector.memset(ones_rowB, 1.0)
        ones_col = persist.tile([128, 1], F32, tag="onesc")
        nc.vector.memset(ones_col, 1.0)
        ones8 = persist.tile([8, 1], F32, tag="ones8")
        nc.vector.memset(ones8, 1.0)
        zero_col = persist.tile([128, 1], F32, tag="zero")
        nc.vector.memset(zero_col, 0.0)
        nc.const_aps.aps[(F32, 0.0)] = zero_col[:, :]
        eps_col = persist.tile([128, 1], F32, tag="eps")
        nc.vector.memset(eps_col, EPS)
        nc.const_aps.aps[(F32, EPS)] = eps_col[:, :]

        outb_col = persist.tile([128, 4], F32, tag="outb")
        nc.sync.dma_start(out=outb_col, in_=obd.rearrange("(i p) -> p i", p=128))
        gng_col = persist.tile([128, 4], F32, tag="gng")
        nc.sync.dma_start(out=gng_col, in_=gngd.rearrange("(i p) -> p i", p=128))
        gnb_col = persist.tile([128, 4], F32, tag="gnb")
        nc.sync.dma_start(out=gnb_col, in_=gnbd.rearrange("(i p) -> p i", p=128))

        # weights (host pre-transposed where needed)
        qwT_sb = persist.tile([128, 4, C], BF16, tag="qwT")
        nc.sync.dma_start(out=qwT_sb, in_=qwTd.rearrange("(i p) o -> p i o", p=128))
        kwT_sb = persist.tile([128, 6, C], BF16, tag="kwT")
        nc.sync.dma_start(out=kwT_sb, in_=kwTd.rearrange("(c p) o -> p c o", p=128))
        vwT_sb = persist.tile([128, 6, C], BF16, tag="vwT")
        nc.sync.dma_start(out=vwT_sb, in_=vwTd.rearrange("(c p) o -> p c o", p=128))

        # row sums of to_k_w / to_v_w (he-layout columns)
        rsk_col = persist.tile([128, 4], F32, tag="rsk")
        rsv_col = persist.tile([128, 4], F32, tag="rsv")
        for dram, col in ((tkd, rsk_col), (tvd, rsv_col)):
            nat = prep.tile([128, 4, C], BF16, tag="wnat")
            nc.sync.dma_start(out=nat, in_=dram.rearrange("(i p) c -> p i c", p=128))
            nc.vector.reduce_sum(out=col, in_=nat, axis=AX)

        # rs_v as a broadcast row scaled by softmax scale
        ps_r = psmp.tile([1, C], F32, tag="pm")
        for ot in range(4):
            nc.tensor.transpose(ps_r[:, ts(ot, 128)], rsv_col[:, ot : ot + 1], ident)
        rsv_row = rowp.tile([1, C], F32, tag="rsvrow")
        nc.vector.tensor_scalar_mul(rsv_row, ps_r, SCALE)
        ps_rb = psmp.tile([128, C], F32, tag="pm")
        nc.tensor.matmul(ps_rb, lhsT=ones_row, rhs=rsv_row, start=True, stop=True)

        # W2 (o-major cols) and its transpose blocks
        ow_nat = prep.tile([128, 4, C], BF16, tag="wnat")
        nc.sync.dma_start(out=ow_nat, in_=owd.rearrange("(i p) c -> p i c", p=128))
        w2c = persist.tile([128, 4, HEADS], F32, tag="w2c")
        for ot in range(4):
            t_ = workp.tile([128, C], F32, tag="tmp")
            nc.vector.tensor_mul(t_, ow_nat[:, ot, :], ps_rb)
            nc.vector.reduce_sum(
                out=w2c[:, ot, :],
                in_=t_.rearrange("p (h d) -> p h d", d=DHEAD),
                axis=AX,
            )
        w2T = persist.tile([HEADS, 4, 128], BF16, tag="w2T")
        for ot in range(4):
            psw = psmp.tile([HEADS, 128], F32, tag="pm")
            nc.tensor.transpose(psw, w2c[:, ot, :], ident)
            nc.vector.tensor_copy(w2T[:, ot, :], psw)
        w2cb = persist.tile([128, 4, HEADS], BF16, tag="w2cb")
        nc.vector.tensor_copy(w2cb, w2c)

        # G = W2^T W2 ; rsW2 = W2^T 1 ; wb = W2^T out_b
        obo4 = persist.tile([128, 4, 2], BF16, tag="obo4")
        nc.vector.memset(obo4, 1.0)
        for ot in range(4):
            nc.vector.tensor_copy(obo4[:, ot, 1:2], outb_col[:, ot : ot + 1])
        ps_rw = psmp.tile([HEADS, 2], F32, tag="pm")
        for ot in range(4):
            nc.tensor.matmul(
                ps_rw, lhsT=w2cb[:, ot, :], rhs=obo4[:, ot, :],
                start=(ot == 0), stop=(ot == 3),
            )
        rwb = persist.tile([HEADS, 2], F32, tag="rwb")
        nc.vector.tensor_copy(rwb, ps_rw)
        ps_g = psmp.tile([HEADS, HEADS], F32, tag="pm")
        for ot in range(4):
            nc.tensor.matmul(
                ps_g, lhsT=w2cb[:, ot, :], rhs=w2cb[:, ot, :],
                start=(ot == 0), stop=(ot == 3),
            )
        Gt = persist.tile([HEADS, HEADS], F32, tag="Gt")
        nc.vector.tensor_copy(Gt, ps_g)

        # sum(out_b), sum(out_b^2) scalars
        ob2 = smallp.tile([128, 4], F32, tag="ob2")
        nc.vector.tensor_mul(ob2, outb_col, outb_col)
        ps_o = psmp.tile([1, 8], F32, tag="pm")
        nc.tensor.matmul(ps_o[:, 0:4], lhsT=ones_col, rhs=outb_col, start=True, stop=True)
        nc.tensor.matmul(ps_o[:, 4:8], lhsT=ones_col, rhs=ob2, start=True, stop=True)
        obsums = rowp.tile([1, 8], F32, tag="obsums")
        nc.vector.tensor_copy(obsums, ps_o)
        obs = persist.tile([1, 2], F32, tag="obs")
        nc.vector.reduce_sum(obs[:, 0:1], obsums[:, 0:4], axis=AX)
        nc.vector.reduce_sum(obs[:, 1:2], obsums[:, 4:8], axis=AX)

        # static head-block mask: Hden[p, ot, h] = 1 iff h == 2*ot + p//64
        Hden = persist.tile([128, 4, HEADS], BF16, tag="Hden")
        nc.vector.memset(Hden, 0.0)
        for ot in range(4):
            nc.vector.memset(Hden[0:64, ot, 2 * ot : 2 * ot + 1], 1.0)
            nc.vector.memset(Hden[64:128, ot, 2 * ot + 1 : 2 * ot + 2], 1.0)

        # ---------------- per-sample pieces ----------------
        def sample_head(s):
            """ky/vy, k-softmax -> w, Mnum mask; returns per-sample tiles."""
            y_col = smallp.tile([128, 6], BF16, tag="ycol")
            nc.sync.dma_start(out=y_col, in_=yd[s].rearrange("(c p) -> p c", p=128))
            rows = {}
            for tag, wsb in (("ky", kwT_sb), ("vy", vwT_sb)):
                ps_k = psmp.tile([1, C], F32, tag="pm")
                for c in range(6):
                    nc.tensor.matmul(
                        ps_k, lhsT=y_col[:, c : c + 1], rhs=wsb[:, c, :],
                        start=(c == 0), stop=(c == 5),
                    )
                r = rowp.tile([1, C], BF16, tag=tag + "row")
                nc.vector.tensor_copy(r, ps_k)
                rows[tag] = r
            ps_vb = psmp.tile([128, C], F32, tag="pm")
            nc.tensor.matmul(ps_vb, lhsT=ones_rowB, rhs=rows["vy"], start=True, stop=True)
            vyb = samp.tile([128, C], F32, tag="vyb")
            nc.vector.tensor_copy(vyb, ps_vb)
            ps_kb = psmp.tile([128, C], F32, tag="pm")
            nc.tensor.matmul(ps_kb, lhsT=ones_rowB, rhs=rows["ky"], start=True, stop=True)
            den4 = samp.tile([128, 4], F32, tag="den4")
            num4 = samp.tile([128, 4], F32, tag="num4")
            for t in range(4):
                ez = ezp.tile([128, C], BF16, tag="ez")
                nc.scalar.activation(
                    out=ez, in_=ps_kb, func=AF.Exp,
                    scale=rsk_col[:, t : t + 1],
                    accum_out=den4[:, t : t + 1],
                )
                scr = ezp.tile([128, C], BF16, tag="scr")
                nc.vector.scalar_tensor_tensor(
                    out=scr, in0=ez, scalar=1.0, in1=vyb,
                    op0=OP.mult, op1=OP.mult,
                    accum_out=num4[:, t : t + 1],
                )
            rcp4 = samp.tile([128, 4], F32, tag="rcp4")
            nc.vector.reciprocal(rcp4, den4)
            wcol = samp.tile([128, 4], F32, tag="wcol")
            nc.vector.tensor_mul(wcol, num4, rcp4)
            Mnum = samp.tile([128, 4, HEADS], BF16, tag="mnum")
            for ot in range(4):
                nc.vector.tensor_scalar_mul(
                    Mnum[:, ot, :], Hden[:, ot, :], wcol[:, ot : ot + 1]
                )
            S2all = samp.tile([HEADS, NG, HEADS], F32, tag="s2all")
            p1a = samp.tile([HEADS, NG], F32, tag="p1a")
            return {"Mnum": Mnum, "S2all": S2all, "p1a": p1a}

        def phase1_g(s, st, g):
            xg = xp.tile([128, 4, GW], BF16, tag="xg")
            nc.sync.dma_start(
                out=xg,
                in_=xd[s].rearrange("(i p) n -> p i n", p=128)[:, :, ts(g, GW)],
            )
            Es = []
            for ot in range(4):
                psq = psqp.tile([128, GW], F32, tag="psq")
                for ct in range(4):
                    nc.tensor.matmul(
                        psq, lhsT=qwT_sb[:, ct, ts(ot, 128)], rhs=xg[:, ct, :],
                        start=(ct == 0), stop=(ct == 3),
                    )
                E = ep.tile([128, GW], BF16, tag="E")
                nc.scalar.activation(out=E, in_=psq, func=AF.Exp)
                Es.append(E)
            ndn = ndnp.tile([HEADS, GW], F32, tag="ndn")
            ndd = nddp.tile([HEADS, GW], F32, tag="ndd")
            for ot in range(4):
                nc.tensor.matmul(
                    ndn, lhsT=st["Mnum"][:, ot, :], rhs=Es[ot],
                    start=(ot == 0), stop=(ot == 3),
                )
                nc.tensor.matmul(
                    ndd, lhsT=Hden[:, ot, :], rhs=Es[ot],
                    start=(ot == 0), stop=(ot == 3),
                )
            rcp8 = rcpp.tile([HEADS, GW], F32, tag="rcp")
            nc.vector.reciprocal_approx_fast(out=rcp8, in_=ndd)
            stt = sttp.tile([HEADS, GW], BF16, tag="stt")
            nc.vector.scalar_tensor_tensor(
                out=stt, in0=ndn, scalar=1.0, in1=rcp8,
                op0=OP.mult, op1=OP.mult,
                accum_out=st["p1a"][:, g : g + 1],
            )
            return stt

        def gram_g(st, stt, g):
            # Gram contribution: transpose s -> [n, h] chunks, S2g = sT.T @ sT
            pst = pstp.tile([128, 32], BF16, tag="pst")
            for j in range(4):
                nc.tensor.transpose(
                    pst[:, ts(j, 8)], stt[:, ts(j, 128)], identB[0:8, 0:8]
                )
            sta = stap.tile([128, 32], BF16, tag="sta")
            nc.vector.tensor_copy(sta, pst)
            psg = psmp.tile([HEADS, HEADS], F32, tag="pm")
            for j in range(4):
                nc.tensor.matmul(
                    psg, lhsT=sta[:, ts(j, 8)], rhs=sta[:, ts(j, 8)],
                    start=(j == 0), stop=(j == 3),
                )
            nc.vector.tensor_copy(st["S2all"][:, g, :], psg)

        def sample_stats(s, st):
            p1 = samp.tile([HEADS, 1], F32, tag="p1")
            nc.vector.reduce_sum(p1, st["p1a"], axis=AX)
            S2s = samp.tile([HEADS, HEADS], F32, tag="s2s")
            nc.vector.reduce_sum(
                S2s, st["S2all"].rearrange("p g h -> p h g"), axis=AX
            )
            tmp3 = samp.tile([HEADS, 3], F32, tag="t3")
            nc.vector.tensor_mul(tmp3[:, 0:1], rwb[:, 0:1], p1)
            nc.vector.tensor_mul(tmp3[:, 2:3], rwb[:, 1:2], p1)
            gs = samp.tile([HEADS, HEADS], F32, tag="gs")
            nc.vector.tensor_mul(gs, Gt, S2s)
            nc.vector.reduce_sum(tmp3[:, 1:2], gs, axis=AX)
            ps_t = psmp.tile([1, 3], F32, tag="pm")
            nc.tensor.matmul(ps_t, lhsT=ones8, rhs=tmp3, start=True, stop=True)
            tt = rowp.tile([1, 12], F32, tag="tt")
            nc.vector.tensor_copy(tt[:, 0:3], ps_t)
            # mu = (sum_mm + N*sum_ob) / M
            nc.vector.scalar_tensor_tensor(
                out=tt[:, 3:4], in0=obs[:, 0:1], scalar=float(N), in1=tt[:, 0:1],
                op0=OP.mult, op1=OP.add,
            )
            nc.vector.tensor_scalar_mul(tt[:, 4:5], tt[:, 3:4], 1.0 / M_TOT)
            # e2 = (sumsq_mm + 2*wb.p1 + N*ssq_ob) / M
            nc.vector.scalar_tensor_tensor(
                out=tt[:, 5:6], in0=tt[:, 2:3], scalar=2.0, in1=tt[:, 1:2],
                op0=OP.mult, op1=OP.add,
            )
            nc.vector.scalar_tensor_tensor(
                out=tt[:, 6:7], in0=obs[:, 1:2], scalar=float(N), in1=tt[:, 5:6],
                op0=OP.mult, op1=OP.add,
            )
            nc.vector.tensor_scalar_mul(tt[:, 7:8], tt[:, 6:7], 1.0 / M_TOT)
            nc.vector.tensor_mul(tt[:, 8:9], tt[:, 4:5], tt[:, 4:5])   # mu^2
            nc.vector.tensor_sub(tt[:, 9:10], tt[:, 7:8], tt[:, 8:9])  # var
            nc.scalar.activation(out=tt[:, 10:11], in_=tt[:, 9:10], func=AF.Sqrt, bias=EPS)
            nc.vector.reciprocal(tt[:, 11:12], tt[:, 10:11])           # rstd
            murow = rowp.tile([1, 2], F32, tag="mur")
            nc.vector.tensor_copy(murow[:, 0:1], tt[:, 4:5])
            nc.vector.tensor_copy(murow[:, 1:2], tt[:, 11:12])
            ps_ms = psmp.tile([128, 2], F32, tag="pm")
            nc.tensor.matmul(ps_ms, lhsT=ones_row, rhs=murow, start=True, stop=True)
            msb = samp.tile([128, 2], F32, tag="msb")
            nc.vector.tensor_copy(msb, ps_ms)
            Acol = samp.tile([128, 4], F32, tag="acol")
            nc.vector.tensor_scalar_mul(Acol, gng_col, msb[:, 1:2])
            tb1 = samp.tile([128, 4], F32, tag="tb1")
            nc.vector.tensor_scalar(
                out=tb1, in0=outb_col, scalar1=msb[:, 0:1], scalar2=None,
                op0=OP.subtract,
            )
            tb2 = samp.tile([128, 4], F32, tag="tb2")
            nc.vector.tensor_mul(tb2, Acol, tb1)
            Bcol = samp.tile([128, 4], F32, tag="bcol")
            nc.vector.tensor_add(Bcol, tb2, gnb_col)
            ps_a = psmp.tile([1, C], F32, tag="pm")
            for ot in range(4):
                nc.tensor.transpose(ps_a[:, ts(ot, 128)], Acol[:, ot : ot + 1], ident)
            a_row = rowp.tile([1, C], F32, tag="arow")
            nc.vector.tensor_copy(a_row, ps_a)
            ps_a8 = psmp.tile([HEADS, C], F32, tag="pm")
            nc.tensor.matmul(
                ps_a8, lhsT=ones_row[:, 0:HEADS], rhs=a_row, start=True, stop=True
            )
            a8 = samp.tile([HEADS, C], F32, tag="a8")
            nc.vector.tensor_copy(a8, ps_a8)
            w2sT = samp.tile([HEADS, 4, 128], BF16, tag="w2s")
            nc.vector.tensor_mul(w2sT, w2T, a8.rearrange("p (i f) -> p i f", i=4))
            return {"w2sT": w2sT, "Bcol": Bcol}

        def phase2_g(s, fin, stt, g):
            stg = stgp.tile([128, 4, GW], BF16, tag="stg")
            for ot in range(4):
                psf = psfp.tile([128, GW], F32, tag="psf")
                nc.tensor.matmul(
                    psf, lhsT=fin["w2sT"][:, ot, :], rhs=stt, start=True, stop=True
                )
                if ot in (0, 2):
                    nc.vector.tensor_scalar_add(
                        stg[:, ot, :], psf, fin["Bcol"][:, ot : ot + 1]
                    )
                else:
                    nc.scalar.activation(
                        out=stg[:, ot, :], in_=psf, func=AF.Identity,
                        bias=fin["Bcol"][:, ot : ot + 1],
                    )
            nc.gpsimd.dma_start(
                out=outd[s].rearrange("(i p) n -> p i n", p=128)[:, :, ts(g, GW)],
                in_=stg,
            )

        # ---------------- schedule ----------------
        # gram_g(g) is deferred one g-block so the PE never waits on the
        # just-divided stt right after its nd matmuls.
        st0 = sample_head(0)
        stt0 = []
        for g in range(NG):
            stt0.append(phase1_g(0, st0, g))
            if g > 0:
                gram_g(st0, stt0[g - 1], g - 1)
        gram_g(st0, stt0[NG - 1], NG - 1)
        fin0 = sample_stats(0, st0)
        st1 = sample_head(1)
        stt1 = []
        for g in range(NG):
            stt1.append(phase1_g(1, st1, g))
            if g > 0:
                gram_g(st1, stt1[g - 1], g - 1)
            phase2_g(0, fin0, stt0[g], g)
        gram_g(st1, stt1[NG - 1], NG - 1)
        fin1 = sample_stats(1, st1)
        for g in range(NG):
            phase2_g(1, fin1, stt1[g], g)

    nc.finalize()
    return nc


_NC_CACHE = {}


def _get_nc(use_f32r=True):
    if use_f32r not in _NC_CACHE:
        _NC_CACHE[use_f32r] = build_nc(use_f32r)
    return _NC_CACHE[use_f32r]


def make_in_maps(inputs):
    x = np.ascontiguousarray(inputs["x"], dtype=np.float32).reshape(B, C, N)
    x = x.astype(NPBF)
    y = np.asarray(inputs["y"], dtype=np.float32).reshape(B, DIMY).astype(NPBF)
    f32 = lambda k: np.asarray(inputs[k], dtype=np.float32)
    shared = {
        "k_wT": f32("k_w").T.astype(NPBF),
        "v_wT": f32("v_w").T.astype(NPBF),
        "to_q_wT": f32("to_q_w").T.astype(NPBF),
        "to_k_w": f32("to_k_w").astype(NPBF),
        "to_v_w": f32("to_v_w").astype(NPBF),
        "out_w": f32("out_w").astype(NPBF),
        "out_b": f32("out_b"),
        "gn_g": f32("gn_g"),
        "gn_b": f32("gn_b"),
    }
    in_maps = []
    for core in range(NCORES):
        s0 = core * BPC
        m = {"x": x[s0 : s0 + BPC], "y": y[s0 : s0 + BPC]}
        m.update(shared)
        in_maps.append(m)
    return in_maps


def kernel(**inputs):
    nc = _get_nc(use_f32r=True)
    res = run_bass_kernel_spmd(nc, make_in_maps(inputs), list(range(NCORES)))
    out = np.concatenate([r["out"] for r in res.results], axis=0)
    return out.astype(np.float32).reshape(B, C, 64, 64)


if __name__ == "__main__":
    rng = np.random.default_rng(0)
    inputs = {
        "x": rng.standard_normal((B, C, 64, 64), dtype=np.float32),
        "y": rng.standard_normal((B, 1, 1, DIMY), dtype=np.float32),
        "k_w": rng.standard_normal((C, DIMY), dtype=np.float32) * 0.02,
        "v_w": rng.standard_normal((C, DIMY), dtype=np.float32) * 0.02,
        "to_q_w": rng.standard_normal((C, C), dtype=np.float32) * 0.02,
        "to_k_w": rng.standard_normal((C, C), dtype=np.float32) * 0.02,
        "to_v_w": rng.standard_normal((C, C), dtype=np.float32) * 0.02,
        "out_w": rng.standard_normal((C, C), dtype=np.float32) * 0.02,
        "out_b": np.zeros(C, np.float32),
        "gn_g": np.ones(C, np.float32),
        "gn_b": np.zeros(C, np.float32),
    }
    out = kernel(**inputs)
    print("kernel ran, out shape", out.shape, "std", out.std())
